# revision 1
# baseline (speedup 1.0000x reference)
"""Distributed attention kernel for 8 trn2 NeuronCores (v3).

Reference semantics (B=2, S=2048, D=2048, H=16, dh=128):
  q = x@W_q, k = x@W_k  (per-head split), v = x@W_v (full width)
  scores = q@k^T per head; (scores + triu(-1e9)) * 1/sqrt(dh); softmax
  out = (sum_h probs_h) @ v @ W_o        <- heads summed, v full width

Sharding: 2 groups of 4 cores (batch parallel); within a group, rank r
owns heads {4r..4r+3} (cols of W_q/W_k), cols [512r, 512r+512) of W_v.
Each core computes P_local = sum of its 4 heads' probs per q-slab
(slab s = q tiles 4s..4s+3, width-trimmed to kw=512(s+1) causal cols).
Per-slab ReduceScatter sums heads and hands rank r its q-tile 4s+r;
PE-transpose to P^T, OT' = P @ v ([q,dv], v AllGathered bf16 in halves),
transpose OT' and Y = OT @ W_o at the end.

Perf notes: collectives are the only gpsimd ops (nothing queues behind
their blocking waits); scores in [128,1024] psum tiles with the causal
mask accumulated on the PE and reduce_max trimmed to the exact causal
width; one batched P-store DMA per slab so each ReduceScatter triggers
promptly; OT'/transposes for slab s issued between C slabs to overlap
the RS ring; W_o's first column half prefetched on the Activation DMA
ring (which paces ~1 DMA/10us, so only these 4 low-urgency loads live
there); streaming loads batched >=512KB on the SP ring. All-core power
capping holds the PE at ~1.2 GHz, so A/B/Y are issued as dense
back-to-back matmul streams across 8 psum banks to pipeline LDWEIGHTS.
"""

import math

import numpy as np
import ml_dtypes

import concourse.bass as bass
import concourse.mybir as mybir
import concourse.tile as tile
from concourse import bacc
from concourse.bass_utils import run_bass_kernel_spmd
from concourse.masks import make_identity

F32 = mybir.dt.float32
F32R = mybir.dt.float32r
BF16 = mybir.dt.bfloat16

S = 2048
D = 2048
DH = 128
NT = S // 128  # 16 q/k tiles
SCALE = 1.0 / math.sqrt(DH)
GROUPS = [[0, 1, 2, 3], [4, 5, 6, 7]]
NEG = -1e9


def build():
    nc = bacc.Bacc("TRN2", target_bir_lowering=False, debug=False, num_devices=8)

    x = nc.declare_dram_parameter("x", [D, S], F32R, isOutput=False)  # x^T
    xbf = nc.declare_dram_parameter("xbf", [D, S], BF16, isOutput=False)
    wq = nc.declare_dram_parameter("wq", [D, 512], F32R, isOutput=False)
    wk = nc.declare_dram_parameter("wk", [D, 512], F32R, isOutput=False)
    wv = nc.declare_dram_parameter("wv", [D, 512], BF16, isOutput=False)
    wo = nc.declare_dram_parameter("wo", [D, D], BF16, isOutput=False)
    out = nc.declare_dram_parameter("out", [512, D], F32, isOutput=True)

    v_local = nc.dram_tensor("v_local", [S, 512], BF16)
    v_ag = [nc.dram_tensor(f"v_ag{h}", [4, 1024, 512], BF16) for h in range(2)]
    # per-slab P partials, width-trimmed to kw = 512*(s+1)
    p_part = [nc.dram_tensor(f"p_part{s}", [512, 512 * (s + 1)], BF16) for s in range(4)]
    p_recv = [nc.dram_tensor(f"p_recv{s}", [128, 512 * (s + 1)], BF16) for s in range(4)]

    with tile.TileContext(nc) as tc:
        # persistent across phases
        qkp = tc.alloc_tile_pool(name="qk", bufs=1)
        qT = qkp.tile([128, 4, S], F32R)  # [dh-part, head, q]
        kT = qkp.tile([128, 4, S], F32R)
        with tc.tile_pool(name="const", bufs=1) as cst:
            ident = cst.tile([128, 128], F32)
            make_identity(nc, ident)
            ident_bf = cst.tile([128, 128], BF16)
            nc.vector.tensor_copy(out=ident_bf[:], in_=ident[:])
            # mask variant m: [128, 512], 0 where col <= row + 128*m else -1e9
            masks = cst.tile([128, 4, 512], BF16)
            for m in range(4):
                nc.gpsimd.memset(masks[:, m, :], 0.0)
                nc.gpsimd.affine_select(
                    out=masks[:, m, :],
                    in_=masks[:, m, :],
                    compare_op=mybir.AluOpType.is_ge,
                    fill=NEG,
                    base=128 * m,
                    pattern=[[-1, 512]],
                    channel_multiplier=1,
                )

            # ---------------- Phase A+B: projections, quarters ----------------
            with (
                tc.tile_pool(name="wsb", bufs=1) as wsb,
                tc.tile_pool(name="xq_pool", bufs=1) as xqp,
                tc.tile_pool(name="xbf_pool", bufs=4) as xbp,
                tc.tile_pool(name="drain", bufs=4) as drp,
                tc.tile_pool(name="ab_ps", bufs=8, space="PSUM") as pjp,
            ):
                wq_sb = wsb.tile([128, NT, 512], F32R)
                wk_sb = wsb.tile([128, NT, 512], F32R)
                wv_sb = wsb.tile([128, NT, 512], BF16)
                # batched preloads, ordered so the first matmuls start early
                wq_src = wq.rearrange("(t p) c -> p t c", p=128)
                wk_src = wk.rearrange("(t p) c -> p t c", p=128)
                wv_src = wv.rearrange("(t p) c -> p t c", p=128)
                x_src = x.rearrange("(t p) s -> p t s", p=128)
                xbf_src = xbf.rearrange("(t p) s -> p t s", p=128)
                nc.sync.dma_start(wq_sb[:, 0:4, :], wq_src[:, 0:4, :])

                for qd in range(4):
                    s0 = qd * 512
                    xq = xqp.tile([128, NT, 512], F32R, tag="xq")
                    for g4 in range(4):
                        nc.sync.dma_start(
                            xq[:, 4 * g4 : 4 * g4 + 4, :],
                            x_src[:, 4 * g4 : 4 * g4 + 4, s0 : s0 + 512],
                        )
                        if qd == 0 and g4 >= 1:
                            nc.sync.dma_start(
                                wq_sb[:, 4 * g4 : 4 * g4 + 4, :],
                                wq_src[:, 4 * g4 : 4 * g4 + 4, :],
                            )
                    if qd == 0:
                        for g4 in range(4):
                            nc.sync.dma_start(
                                wk_sb[:, 4 * g4 : 4 * g4 + 4, :],
                                wk_src[:, 4 * g4 : 4 * g4 + 4, :],
                            )
                        for g4 in range(4):
                            nc.sync.dma_start(
                                wv_sb[:, 4 * g4 : 4 * g4 + 4, :],
                                wv_src[:, 4 * g4 : 4 * g4 + 4, :],
                            )
                    for dst, wsrc, eng in ((qT, wq_sb, "s"), (kT, wk_sb, "v")):
                        psums = [
                            pjp.tile([128, 512], F32, tag="ps", name=f"pj{_j}")
                            for _j in range(4)
                        ]
                        for Dt in range(NT):
                            for dt in range(4):
                                nc.tensor.matmul(
                                    psums[dt][:],
                                    wsrc[:, Dt, dt * 128 : (dt + 1) * 128],
                                    xq[:, Dt, :],
                                    start=(Dt == 0),
                                    stop=(Dt == NT - 1),
                                )
                        for dt in range(4):
                            if eng == "s":
                                nc.scalar.copy(
                                    out=dst[:, dt, s0 : s0 + 512], in_=psums[dt][:]
                                )
                            else:
                                nc.vector.tensor_copy(
                                    out=dst[:, dt, s0 : s0 + 512], in_=psums[dt][:]
                                )
                    # v-pass (bf16; xbf streamed; covers next-quarter xq WAR window)
                    psums = [
                        pjp.tile([128, 512], F32, tag="ps", name=f"pv{_j}")
                        for _j in range(4)
                    ]
                    for g4 in range(4):
                        xb_t = xbp.tile([128, 4, 512], BF16, tag="xb")
                        nc.sync.dma_start(
                            xb_t[:], xbf_src[:, 4 * g4 : 4 * g4 + 4, s0 : s0 + 512]
                        )
                        for dj in range(4):
                            Dt = 4 * g4 + dj
                            for sb in range(4):
                                nc.tensor.matmul(
                                    psums[sb][:],
                                    xb_t[:, dj, sb * 128 : (sb + 1) * 128],
                                    wv_sb[:, Dt, :],
                                    start=(Dt == 0),
                                    stop=(Dt == NT - 1),
                                )
                    for sb in range(4):
                        v_sb = drp.tile([128, 512], BF16, tag="vsb")
                        nc.vector.tensor_copy(out=v_sb[:], in_=psums[sb][:])
                        r0 = s0 + sb * 128
                        nc.sync.dma_start(v_local[r0 : r0 + 128, :], v_sb[:])
                    if qd == 1 or qd == 3:
                        h = qd // 2
                        nc.gpsimd.collective_compute(
                            "AllGather",
                            mybir.AluOpType.bypass,
                            ins=[v_local[h * 1024 : (h + 1) * 1024, :]],
                            outs=[v_ag[h][:]],
                            replica_groups=GROUPS,
                        )

            # ---------------- Phase C + D interleaved ----------------
            otp = tc.alloc_tile_pool(name="otp", bufs=1, side="right")
            ot = otp.tile([128, NT, 512], BF16)    # [dv-part, dvt, own-q] for Y
            ot_q = otp.tile([128, 4, 2048], BF16)  # [own-q-part, slab, dv] from OT'
            ep = tc.alloc_tile_pool(name="epool", bufs=2, side="right")
            with (
                tc.tile_pool(name="small", bufs=48) as smp,
                tc.tile_pool(name="dsm", bufs=8) as dsm,
                tc.tile_pool(name="psb", bufs=1) as psbp,
                tc.tile_pool(name="rp", bufs=1) as rp,
                tc.tile_pool(name="ptp", bufs=2) as ptp,
                tc.tile_pool(name="vfp", bufs=2) as vfp,
                tc.tile_pool(name="sc_ps", bufs=3, space="PSUM") as scp,
                tc.tile_pool(name="p_ps", bufs=1, space="PSUM") as ppp,
                tc.tile_pool(name="tr_ps", bufs=1, space="PSUM") as trp,
            ):
                def issue_scores(i):
                    """Score matmuls for q-tile i, all 4 heads -> [128,1024] tiles.
                    The causal mask is accumulated on the PE (identity @ mask)."""
                    s = i // 4
                    kwc = s + 1                    # 512-chunks
                    ntile = (kwc + 1) // 2         # 1024-psum tiles
                    m0 = (i % 4) * 128
                    hctx = []
                    for h in range(4):
                        s_tiles = [
                            scp.tile([128, 1024], F32, tag="S", name=f"sc{i}h{h}t{_t}")
                            for _t in range(ntile)
                        ]
                        for kc in range(kwc):
                            diag = kc == kwc - 1
                            tgt = s_tiles[kc // 2][:, (kc % 2) * 512 : (kc % 2) * 512 + 512]
                            nc.tensor.matmul(
                                tgt,
                                qT[:, h, i * 128 : (i + 1) * 128],
                                kT[:, h, kc * 512 : (kc + 1) * 512],
                                start=True,
                                stop=not diag,
                            )
                            if diag:
                                nc.tensor.matmul(
                                    tgt[:, m0:512],
                                    ident_bf[:],
                                    masks[:, i % 4, m0:512],
                                    start=False,
                                    stop=True,
                                )
                        hctx.append(s_tiles)
                    return (i, kwc, ntile, hctx)

                def issue_softmax(ctx, e_t):
                    """Trimmed max, full-kw exp, batched rinv for all heads of i."""
                    i, kwc, ntile, hctx = ctx
                    kw = kwc * 512
                    vw = 128 * (i + 1)             # exact causal width
                    rt4 = smp.tile([128, 4], F32, tag="rt4")
                    for h in range(4):
                        s_tiles = hctx[h]
                        mx = None
                        for t in range(ntile):
                            w = min(vw - 1024 * t, 1024)
                            if w <= 0:
                                break
                            mxt = smp.tile([128, 1], F32, tag="mx")
                            nc.vector.reduce_max(
                                out=mxt[:],
                                in_=s_tiles[t][:, :w],
                                axis=mybir.AxisListType.X,
                            )
                            if mx is None:
                                mx = mxt
                            else:
                                mxn = smp.tile([128, 1], F32, tag="mx")
                                nc.vector.tensor_tensor(
                                    out=mxn[:], in0=mx[:], in1=mxt[:],
                                    op=mybir.AluOpType.max,
                                )
                                mx = mxn
                        nmS = smp.tile([128, 1], F32, tag="mx")
                        nc.vector.tensor_scalar_mul(nmS[:], mx[:], -SCALE)
                        rcs = []
                        for t in range(ntile):
                            w = min(kw - 1024 * t, 1024)
                            if ntile == 1:
                                rc_ap = rt4[:, h : h + 1]
                            else:
                                rc_t = smp.tile([128, 1], F32, tag="mx", name=f"rc{t}")
                                rc_ap = rc_t[:]
                            nc.scalar.activation(
                                out=e_t[:, h, 1024 * t : 1024 * t + w],
                                in_=s_tiles[t][:, :w],
                                func=mybir.ActivationFunctionType.Exp,
                                bias=nmS[:],
                                scale=SCALE,
                                accum_out=rc_ap,
                            )
                            rcs.append(rc_ap)
                        if ntile == 2:
                            nc.vector.tensor_tensor(
                                out=rt4[:, h : h + 1], in0=rcs[0], in1=rcs[1],
                                op=mybir.AluOpType.add,
                            )
                    ri4 = smp.tile([128, 4], F32, tag="rt4")
                    nc.vector.reciprocal(out=ri4[:], in_=rt4[:])
                    d_hs = []
                    for h in range(4):
                        d_h = dsm.tile([128, 128], BF16, tag="D")
                        nc.vector.tensor_scalar_mul(
                            d_h[:], ident_bf[:], ri4[:, h : h + 1]
                        )
                        d_hs.append(d_h)
                    return d_hs

                def issue_p(ctx, e_t, d_hs, psl):
                    i, kwc, ntile, hctx = ctx
                    s = i // 4
                    for kc in range(kwc):
                        p_t = ppp.tile([128, 512], F32, tag="P")
                        for h in range(4):
                            nc.tensor.matmul(
                                p_t[:],
                                d_hs[h][:],
                                e_t[:, h, kc * 512 : (kc + 1) * 512],
                                start=(h == 0),
                                stop=(h == 3),
                            )
                        if kc % 2 == 0:
                            nc.scalar.copy(
                                out=psl[:, i - 4 * s, kc * 512 : (kc + 1) * 512],
                                in_=p_t[:],
                            )
                        else:
                            nc.vector.tensor_copy(
                                out=psl[:, i - 4 * s, kc * 512 : (kc + 1) * 512],
                                in_=p_t[:],
                            )

                def issue_slab_C(s):
                    """Scores+softmax+P for slab s with 1-deep pipeline."""
                    kw = 512 * (s + 1)
                    psl = psbp.tile([128, 4, 2048], BF16, tag="psb", name=f"psl{s}")
                    pend = None
                    for i in range(4 * s, 4 * s + 4):
                        ctx = issue_scores(i)
                        if pend is not None:
                            issue_p(*pend, psl)
                            pend = None
                        e_t = ep.tile([128, 4, 2048], BF16, tag="E")
                        d_hs = issue_softmax(ctx, e_t)
                        pend = (ctx, e_t, d_hs)
                    issue_p(*pend, psl)
                    nc.sync.dma_start(
                        p_part[s].rearrange("(i p) k -> p i k", p=128),
                        psl[:, :, :kw],
                    )
                    nc.gpsimd.collective_compute(
                        "ReduceScatter",
                        mybir.AluOpType.add,
                        ins=[p_part[s][:]],
                        outs=[p_recv[s][:]],
                        replica_groups=GROUPS,
                    )

                def issue_slab_D(s):
                    """Load P_own, transpose to pt, OT' = P@v, transpose to ot."""
                    kw = 512 * (s + 1)
                    nkt = 4 * (s + 1)
                    pown = rp.tile([128, 2048], BF16, tag="POW")
                    nc.sync.dma_start(pown[:, :kw], p_recv[s][:])
                    # pt[k-part, kt, q] = P_own^T via PE transposes (8 per bank-tile)
                    pt = ptp.tile([128, NT, 128], BF16, tag="PT")
                    for kg in range((nkt + 7) // 8):
                        nsl = min(nkt - 8 * kg, 8)
                        tr = trp.tile([128, 8, 128], BF16, tag="TR", name=f"tr{s}_{kg}")
                        for j in range(nsl):
                            kt = 8 * kg + j
                            nc.tensor.transpose(
                                tr[:, j, :],
                                pown[:, kt * 128 : (kt + 1) * 128],
                                ident_bf[:],
                            )
                        nc.vector.tensor_copy(
                            out=pt[:, 8 * kg : 8 * kg + nsl, :], in_=tr[:, :nsl, :]
                        )
                    # OT'[q, dv] = sum_kt pt[kt]^T? no: lhsT=pt[kt] ([k,q]) -> out [q, dv]
                    for half in range(2):
                        c0 = half * 1024
                        po = scp.tile([128, 1024], F32, tag="S", name=f"po{s}_{half}")
                        for kg in range(nkt // 4):  # 4 k-tiles per vf load
                            vf = vfp.tile([128, 4, 1024], BF16, tag="VF")
                            hh = kg // 2
                            t0 = (4 * kg) % 8
                            for gg in range(2):
                                vsrc = v_ag[hh][2 * half + gg].rearrange(
                                    "(t p) d -> p t d", p=128
                                )
                                nc.sync.dma_start(
                                    vf[:, :, gg * 512 : (gg + 1) * 512],
                                    vsrc[:, t0 : t0 + 4, :],
                                )
                            for kj in range(4):
                                kt = 4 * kg + kj
                                for sub in range(2):
                                    nc.tensor.matmul(
                                        po[:, sub * 512 : (sub + 1) * 512],
                                        pt[:, kt, :],
                                        vf[:, kj, sub * 512 : (sub + 1) * 512],
                                        start=(kt == 0),
                                        stop=(kt == nkt - 1),
                                    )
                        if half == 0:
                            nc.scalar.copy(
                                out=ot_q[:, s, c0 : c0 + 1024], in_=po[:]
                            )
                        else:
                            nc.vector.tensor_copy(
                                out=ot_q[:, s, c0 : c0 + 1024], in_=po[:]
                            )
                    # transpose OT' [q, dv] -> ot [dv, q] for the Y phase
                    for dg in range(2):
                        tr = trp.tile([128, 8, 128], BF16, tag="TR", name=f"ot{s}_{dg}")
                        for j in range(8):
                            dvt = 8 * dg + j
                            nc.tensor.transpose(
                                tr[:, j, :],
                                ot_q[:, s, dvt * 128 : (dvt + 1) * 128],
                                ident_bf[:],
                            )
                        if dg == 0:
                            nc.scalar.copy(
                                out=ot[:, 0:8, s * 128 : (s + 1) * 128], in_=tr[:]
                            )
                        else:
                            nc.vector.tensor_copy(
                                out=ot[:, 8:16, s * 128 : (s + 1) * 128], in_=tr[:]
                            )

                issue_slab_C(3)
                issue_slab_C(2)
                issue_slab_C(1)
                issue_slab_D(3)
                issue_slab_C(0)
                ep.release()
                wopre = tc.alloc_tile_pool(name="wopre", bufs=1, side="right")
                wo_sb = wopre.tile([128, NT, 1024], BF16)
                wo_src0 = wo.rearrange("(t p) c -> p t c", p=128)
                for g4 in range(4):
                    nc.scalar.dma_start(
                        wo_sb[:, 4 * g4 : 4 * g4 + 4, :],
                        wo_src0[:, 4 * g4 : 4 * g4 + 4, 0:1024],
                    )
                issue_slab_D(2)
                issue_slab_D(1)
                issue_slab_D(0)

        # ---------------- Phase Y ----------------
        with (
            tc.tile_pool(name="wop", bufs=4) as wop,
            tc.tile_pool(name="ysb", bufs=4) as ysbp,
            tc.tile_pool(name="y_ps", bufs=8, space="PSUM") as yps,
        ):
            for nh in range(2):  # output col halves [0,1024), [1024,2048)
                c0 = nh * 1024
                yp = [
                    yps.tile([128, 512], F32, tag="Y", name=f"y{nh}_{_j}")
                    for _j in range(8)
                ]
                wo_src = wo.rearrange("(t p) c -> p t c", p=128)
                for g4 in range(4):
                    if nh == 0:
                        wo_t = wo_sb[:, 4 * g4 : 4 * g4 + 4, :]
                    else:
                        wo_tile = wop.tile([128, 4, 1024], BF16, tag="wo")
                        nc.scalar.dma_start(
                            wo_tile[:],
                            wo_src[:, 4 * g4 : 4 * g4 + 4, c0 : c0 + 1024],
                        )
                        wo_t = wo_tile[:]
                    for dj in range(4):
                        dvt = 4 * g4 + dj
                        for j in range(8):
                            qb, nc2 = divmod(j, 2)
                            nc.tensor.matmul(
                                yp[j][:],
                                ot[:, dvt, qb * 128 : (qb + 1) * 128],
                                wo_t[:, dj, nc2 * 512 : (nc2 + 1) * 512],
                                start=(dvt == 0),
                                stop=(dvt == NT - 1),
                            )
                for j in range(8):
                    qb, nc2 = divmod(j, 2)
                    y_sb = ysbp.tile([128, 512], F32, tag="ysb")
                    if j % 2 == 0:
                        nc.scalar.copy(out=y_sb[:], in_=yp[j][:])
                    else:
                        nc.vector.tensor_copy(out=y_sb[:], in_=yp[j][:])
                    nc.sync.dma_start(
                        out[
                            qb * 128 : (qb + 1) * 128,
                            c0 + nc2 * 512 : c0 + nc2 * 512 + 512,
                        ],
                        y_sb[:],
                    )
        wopre.release()
        otp.release()
        qkp.release()

    nc.compile()
    return nc


_NC_CACHE = None


def kernel(x, W_q, W_k, W_v, W_o):
    global _NC_CACHE
    x = np.asarray(x, dtype=np.float32)
    W_q = np.asarray(W_q, dtype=np.float32)
    W_k = np.asarray(W_k, dtype=np.float32)
    W_v = np.asarray(W_v, dtype=np.float32)
    W_o = np.asarray(W_o, dtype=np.float32)
    if _NC_CACHE is None:
        _NC_CACHE = build()
    nc = _NC_CACHE

    wo_bf = W_o.astype(ml_dtypes.bfloat16)
    xT = [np.ascontiguousarray(x[g].T) for g in range(2)]
    xT_bf = [t.astype(ml_dtypes.bfloat16) for t in xT]
    in_maps = []
    for c in range(8):
        g, r = divmod(c, 4)
        in_maps.append(
            {
                "x": xT[g],
                "xbf": xT_bf[g],
                "wq": np.ascontiguousarray(W_q[:, 512 * r : 512 * (r + 1)]),
                "wk": np.ascontiguousarray(W_k[:, 512 * r : 512 * (r + 1)]),
                "wv": np.ascontiguousarray(W_v[:, 512 * r : 512 * (r + 1)]).astype(ml_dtypes.bfloat16),
                "wo": wo_bf,
            }
        )
    res = run_bass_kernel_spmd(nc, in_maps, core_ids=list(range(8)))
    Y = np.empty((2, S, D), dtype=np.float32)
    for c in range(8):
        g, r = divmod(c, 4)
        o = res.results[c]["out"]
        for s_idx in range(4):
            t = 4 * s_idx + r
            Y[g, t * 128 : (t + 1) * 128, :] = o[s_idx * 128 : (s_idx + 1) * 128, :]
    return Y



# revision 2
# speedup vs baseline: 1.1687x; 1.1687x over previous
"""Distributed attention kernel for 8 trn2 NeuronCores (v3).

Reference semantics (B=2, S=2048, D=2048, H=16, dh=128):
  q = x@W_q, k = x@W_k  (per-head split), v = x@W_v (full width)
  scores = q@k^T per head; (scores + triu(-1e9)) * 1/sqrt(dh); softmax
  out = (sum_h probs_h) @ v @ W_o        <- heads summed, v full width

Key algebraic fold: out = P @ (x @ (W_v @ W_o)) = P @ U with
U = x @ Wvo precomputed host-side (fp32) — removes the final Y=OT@W_o
matmul phase entirely (PE cycles AND its serialized tail after the
last ReduceScatter).

Sharding: 2 groups of 4 cores (batch parallel); within a group, rank r
owns heads {4r..4r+3} (cols of W_q/W_k), cols [512r, 512r+512) of Wvo.
Each core computes P_local = sum of its 4 heads' probs per q-slab
(slab s = q tiles 4s..4s+3, width-trimmed to kw=512(s+1) causal cols).
Per-slab ReduceScatter sums heads and hands rank r its q-tile 4s+r;
PE-transpose to P^T, OUT = P @ U ([q,dv], U AllGathered bf16 in
halves) written straight to the output tensor.

Perf notes: collectives are the only gpsimd ops (nothing queues behind
their blocking waits); scores in [128,1024] psum tiles with the causal
mask accumulated on the PE and reduce_max trimmed to the exact causal
width; per-q-tile P-store DMAs so each ReduceScatter triggers as soon
as its last tile lands; C slabs run 3,2,1,0 back-to-back so the PE
never waits on the RS ring (D slabs follow, each overlapping the next
RS); streaming loads batched >=512KB on the SP ring. A/B issued as
dense back-to-back matmul streams across 8 psum banks.
"""

import math

import numpy as np
import ml_dtypes

import concourse.bass as bass
import concourse.mybir as mybir
import concourse.tile as tile
from concourse import bacc
from concourse.bass_utils import run_bass_kernel_spmd
from concourse.masks import make_identity

F32 = mybir.dt.float32
F32R = mybir.dt.float32r
BF16 = mybir.dt.bfloat16

S = 2048
D = 2048
DH = 128
NT = S // 128  # 16 q/k tiles
SCALE = 1.0 / math.sqrt(DH)
GROUPS = [[0, 1, 2, 3], [4, 5, 6, 7]]
NEG = -1e9


def build():
    nc = bacc.Bacc("TRN2", target_bir_lowering=False, debug=False, num_devices=8)

    x = nc.declare_dram_parameter("x", [D, S], F32R, isOutput=False)  # x^T
    xbf = nc.declare_dram_parameter("xbf", [D, S], BF16, isOutput=False)
    wq = nc.declare_dram_parameter("wq", [D, 512], F32R, isOutput=False)
    wk = nc.declare_dram_parameter("wk", [D, 512], F32R, isOutput=False)
    wv = nc.declare_dram_parameter("wv", [D, 512], BF16, isOutput=False)  # Wvo slice
    out = nc.declare_dram_parameter("out", [512, D], F32, isOutput=True)

    v_local = nc.dram_tensor("v_local", [S, 512], BF16)
    v_ag = [nc.dram_tensor(f"v_ag{h}", [4, 1024, 512], BF16) for h in range(2)]
    # per-slab P partials, width-trimmed to kw = 512*(s+1)
    p_part = [nc.dram_tensor(f"p_part{s}", [512, 512 * (s + 1)], BF16) for s in range(4)]
    p_recv = [nc.dram_tensor(f"p_recv{s}", [128, 512 * (s + 1)], BF16) for s in range(4)]

    with tile.TileContext(nc) as tc:
        # persistent across phases
        qkp = tc.alloc_tile_pool(name="qk", bufs=1)
        qT = qkp.tile([128, 4, S], F32R)  # [dh-part, head, q]
        kT = qkp.tile([128, 4, S], F32R)
        with tc.tile_pool(name="const", bufs=1) as cst:
            ident = cst.tile([128, 128], F32)
            make_identity(nc, ident)
            ident_bf = cst.tile([128, 128], BF16)
            nc.vector.tensor_copy(out=ident_bf[:], in_=ident[:])
            # mask variant m: [128, 512], 0 where col <= row + 128*m else -1e9
            masks = cst.tile([128, 4, 512], BF16)
            for m in range(4):
                nc.gpsimd.memset(masks[:, m, :], 0.0)
                nc.gpsimd.affine_select(
                    out=masks[:, m, :],
                    in_=masks[:, m, :],
                    compare_op=mybir.AluOpType.is_ge,
                    fill=NEG,
                    base=128 * m,
                    pattern=[[-1, 512]],
                    channel_multiplier=1,
                )

            # ---------------- Phase A+B: projections, quarters ----------------
            with (
                tc.tile_pool(name="wsb", bufs=1) as wsb,
                tc.tile_pool(name="xq_pool", bufs=1) as xqp,
                tc.tile_pool(name="xbf_pool", bufs=4) as xbp,
                tc.tile_pool(name="drain", bufs=4) as drp,
                tc.tile_pool(name="ab_ps", bufs=8, space="PSUM") as pjp,
            ):
                wq_sb = wsb.tile([128, NT, 512], F32R)
                wk_sb = wsb.tile([128, NT, 512], F32R)
                wv_sb = wsb.tile([128, NT, 512], BF16)
                # batched preloads, ordered so the first matmuls start early
                wq_src = wq.rearrange("(t p) c -> p t c", p=128)
                wk_src = wk.rearrange("(t p) c -> p t c", p=128)
                wv_src = wv.rearrange("(t p) c -> p t c", p=128)
                x_src = x.rearrange("(t p) s -> p t s", p=128)
                xbf_src = xbf.rearrange("(t p) s -> p t s", p=128)
                nc.sync.dma_start(wq_sb[:, 0:4, :], wq_src[:, 0:4, :])

                for qd in range(4):
                    s0 = qd * 512
                    xq = xqp.tile([128, NT, 512], F32R, tag="xq")
                    for g4 in range(4):
                        nc.sync.dma_start(
                            xq[:, 4 * g4 : 4 * g4 + 4, :],
                            x_src[:, 4 * g4 : 4 * g4 + 4, s0 : s0 + 512],
                        )
                        if qd == 0 and g4 >= 1:
                            nc.sync.dma_start(
                                wq_sb[:, 4 * g4 : 4 * g4 + 4, :],
                                wq_src[:, 4 * g4 : 4 * g4 + 4, :],
                            )
                    if qd == 0:
                        for g4 in range(4):
                            nc.sync.dma_start(
                                wk_sb[:, 4 * g4 : 4 * g4 + 4, :],
                                wk_src[:, 4 * g4 : 4 * g4 + 4, :],
                            )
                        for g4 in range(4):
                            nc.sync.dma_start(
                                wv_sb[:, 4 * g4 : 4 * g4 + 4, :],
                                wv_src[:, 4 * g4 : 4 * g4 + 4, :],
                            )
                    for dst, wsrc, eng in ((qT, wq_sb, "s"), (kT, wk_sb, "v")):
                        psums = [
                            pjp.tile([128, 512], F32, tag="ps", name=f"pj{_j}")
                            for _j in range(4)
                        ]
                        for Dt in range(NT):
                            for dt in range(4):
                                nc.tensor.matmul(
                                    psums[dt][:],
                                    wsrc[:, Dt, dt * 128 : (dt + 1) * 128],
                                    xq[:, Dt, :],
                                    start=(Dt == 0),
                                    stop=(Dt == NT - 1),
                                )
                        for dt in range(4):
                            if eng == "s":
                                nc.scalar.copy(
                                    out=dst[:, dt, s0 : s0 + 512], in_=psums[dt][:]
                                )
                            else:
                                nc.vector.tensor_copy(
                                    out=dst[:, dt, s0 : s0 + 512], in_=psums[dt][:]
                                )
                    # v-pass (bf16; xbf streamed; covers next-quarter xq WAR window)
                    psums = [
                        pjp.tile([128, 512], F32, tag="ps", name=f"pv{_j}")
                        for _j in range(4)
                    ]
                    for g4 in range(4):
                        xb_t = xbp.tile([128, 4, 512], BF16, tag="xb")
                        nc.sync.dma_start(
                            xb_t[:], xbf_src[:, 4 * g4 : 4 * g4 + 4, s0 : s0 + 512]
                        )
                        for dj in range(4):
                            Dt = 4 * g4 + dj
                            for sb in range(4):
                                nc.tensor.matmul(
                                    psums[sb][:],
                                    xb_t[:, dj, sb * 128 : (sb + 1) * 128],
                                    wv_sb[:, Dt, :],
                                    start=(Dt == 0),
                                    stop=(Dt == NT - 1),
                                )
                    for sb in range(4):
                        v_sb = drp.tile([128, 512], BF16, tag="vsb")
                        nc.vector.tensor_copy(out=v_sb[:], in_=psums[sb][:])
                        r0 = s0 + sb * 128
                        nc.sync.dma_start(v_local[r0 : r0 + 128, :], v_sb[:])
                    if qd == 1 or qd == 3:
                        h = qd // 2
                        nc.gpsimd.collective_compute(
                            "AllGather",
                            mybir.AluOpType.bypass,
                            ins=[v_local[h * 1024 : (h + 1) * 1024, :]],
                            outs=[v_ag[h][:]],
                            replica_groups=GROUPS,
                        )

            # ---------------- Phase C (all slabs) then D ----------------
            ep = tc.alloc_tile_pool(name="epool", bufs=2, side="right")
            with (
                tc.tile_pool(name="small", bufs=48) as smp,
                tc.tile_pool(name="dsm", bufs=8) as dsm,
                tc.tile_pool(name="psb", bufs=1) as psbp,
                tc.tile_pool(name="rp", bufs=1) as rp,
                tc.tile_pool(name="ptp", bufs=2) as ptp,
                tc.tile_pool(name="vfp", bufs=2) as vfp,
                tc.tile_pool(name="ysb", bufs=4) as ysbp,
                tc.tile_pool(name="sc_ps", bufs=3, space="PSUM") as scp,
                tc.tile_pool(name="p_ps", bufs=1, space="PSUM") as ppp,
                tc.tile_pool(name="tr_ps", bufs=1, space="PSUM") as trp,
            ):
                def issue_scores(i):
                    """Score matmuls for q-tile i, all 4 heads -> [128,1024] tiles.
                    The causal mask is accumulated on the PE (identity @ mask)."""
                    s = i // 4
                    kwc = s + 1                    # 512-chunks
                    ntile = (kwc + 1) // 2         # 1024-psum tiles
                    m0 = (i % 4) * 128
                    hctx = []
                    for h in range(4):
                        s_tiles = [
                            scp.tile([128, 1024], F32, tag="S", name=f"sc{i}h{h}t{_t}")
                            for _t in range(ntile)
                        ]
                        for kc in range(kwc):
                            diag = kc == kwc - 1
                            tgt = s_tiles[kc // 2][:, (kc % 2) * 512 : (kc % 2) * 512 + 512]
                            nc.tensor.matmul(
                                tgt,
                                qT[:, h, i * 128 : (i + 1) * 128],
                                kT[:, h, kc * 512 : (kc + 1) * 512],
                                start=True,
                                stop=not diag,
                            )
                            if diag:
                                nc.tensor.matmul(
                                    tgt[:, m0:512],
                                    ident_bf[:],
                                    masks[:, i % 4, m0:512],
                                    start=False,
                                    stop=True,
                                )
                        hctx.append(s_tiles)
                    return (i, kwc, ntile, hctx)

                def issue_softmax(ctx, e_t):
                    """Trimmed max, full-kw exp, batched rinv for all heads of i."""
                    i, kwc, ntile, hctx = ctx
                    kw = kwc * 512
                    vw = 128 * (i + 1)             # exact causal width
                    rt4 = smp.tile([128, 4], F32, tag="rt4")
                    for h in range(4):
                        s_tiles = hctx[h]
                        mx = None
                        for t in range(ntile):
                            w = min(vw - 1024 * t, 1024)
                            if w <= 0:
                                break
                            mxt = smp.tile([128, 1], F32, tag="mx")
                            nc.vector.reduce_max(
                                out=mxt[:],
                                in_=s_tiles[t][:, :w],
                                axis=mybir.AxisListType.X,
                            )
                            if mx is None:
                                mx = mxt
                            else:
                                mxn = smp.tile([128, 1], F32, tag="mx")
                                nc.vector.tensor_tensor(
                                    out=mxn[:], in0=mx[:], in1=mxt[:],
                                    op=mybir.AluOpType.max,
                                )
                                mx = mxn
                        nmS = smp.tile([128, 1], F32, tag="mx")
                        nc.vector.tensor_scalar_mul(nmS[:], mx[:], -SCALE)
                        rcs = []
                        for t in range(ntile):
                            w = min(kw - 1024 * t, 1024)
                            if ntile == 1:
                                rc_ap = rt4[:, h : h + 1]
                            else:
                                rc_t = smp.tile([128, 1], F32, tag="mx", name=f"rc{t}")
                                rc_ap = rc_t[:]
                            nc.scalar.activation(
                                out=e_t[:, h, 1024 * t : 1024 * t + w],
                                in_=s_tiles[t][:, :w],
                                func=mybir.ActivationFunctionType.Exp,
                                bias=nmS[:],
                                scale=SCALE,
                                accum_out=rc_ap,
                            )
                            rcs.append(rc_ap)
                        if ntile == 2:
                            nc.vector.tensor_tensor(
                                out=rt4[:, h : h + 1], in0=rcs[0], in1=rcs[1],
                                op=mybir.AluOpType.add,
                            )
                    ri4 = smp.tile([128, 4], F32, tag="rt4")
                    nc.vector.reciprocal(out=ri4[:], in_=rt4[:])
                    d_hs = []
                    for h in range(4):
                        d_h = dsm.tile([128, 128], BF16, tag="D")
                        nc.vector.tensor_scalar_mul(
                            d_h[:], ident_bf[:], ri4[:, h : h + 1]
                        )
                        d_hs.append(d_h)
                    return d_hs

                def issue_p(ctx, e_t, d_hs, psl):
                    i, kwc, ntile, hctx = ctx
                    s = i // 4
                    il = i - 4 * s
                    kw = kwc * 512
                    for kc in range(kwc):
                        p_t = ppp.tile([128, 512], F32, tag="P")
                        for h in range(4):
                            nc.tensor.matmul(
                                p_t[:],
                                d_hs[h][:],
                                e_t[:, h, kc * 512 : (kc + 1) * 512],
                                start=(h == 0),
                                stop=(h == 3),
                            )
                        if kc % 2 == 0:
                            nc.scalar.copy(
                                out=psl[:, il, kc * 512 : (kc + 1) * 512],
                                in_=p_t[:],
                            )
                        else:
                            nc.vector.tensor_copy(
                                out=psl[:, il, kc * 512 : (kc + 1) * 512],
                                in_=p_t[:],
                            )
                    # per-tile store so the RS can trigger as soon as the
                    # last tile of the slab lands
                    nc.sync.dma_start(
                        p_part[s][il * 128 : (il + 1) * 128, :],
                        psl[:, il, :kw],
                    )

                def issue_slab_C(s):
                    """Scores+softmax+P for slab s with 1-deep pipeline."""
                    psl = psbp.tile([128, 4, 2048], BF16, tag="psb", name=f"psl{s}")
                    pend = None
                    for i in range(4 * s, 4 * s + 4):
                        ctx = issue_scores(i)
                        if pend is not None:
                            issue_p(*pend, psl)
                            pend = None
                        e_t = ep.tile([128, 4, 2048], BF16, tag="E")
                        d_hs = issue_softmax(ctx, e_t)
                        pend = (ctx, e_t, d_hs)
                    issue_p(*pend, psl)
                    nc.gpsimd.collective_compute(
                        "ReduceScatter",
                        mybir.AluOpType.add,
                        ins=[p_part[s][:]],
                        outs=[p_recv[s][:]],
                        replica_groups=GROUPS,
                    )

                def issue_slab_D(s):
                    """Load P_own, transpose to pt, OUT = P@U -> dram out."""
                    kw = 512 * (s + 1)
                    nkt = 4 * (s + 1)
                    pown = rp.tile([128, 2048], BF16, tag="POW")
                    nc.sync.dma_start(pown[:, :kw], p_recv[s][:])
                    # pt[k-part, kt, q] = P_own^T via PE transposes (8 per bank-tile)
                    pt = ptp.tile([128, NT, 128], BF16, tag="PT")
                    for kg in range((nkt + 7) // 8):
                        nsl = min(nkt - 8 * kg, 8)
                        tr = trp.tile([128, 8, 128], BF16, tag="TR", name=f"tr{s}_{kg}")
                        for j in range(nsl):
                            kt = 8 * kg + j
                            nc.tensor.transpose(
                                tr[:, j, :],
                                pown[:, kt * 128 : (kt + 1) * 128],
                                ident_bf[:],
                            )
                        nc.vector.tensor_copy(
                            out=pt[:, 8 * kg : 8 * kg + nsl, :], in_=tr[:, :nsl, :]
                        )
                    # OUT[q, dv] = sum_kt pt[kt].T @ U[kt]  (U = v_ag, bf16)
                    for half in range(2):
                        c0 = half * 1024
                        po = scp.tile([128, 1024], F32, tag="S", name=f"po{s}_{half}")
                        for kg in range(nkt // 4):  # 4 k-tiles per vf load
                            vf = vfp.tile([128, 4, 1024], BF16, tag="VF")
                            hh = kg // 2
                            t0 = (4 * kg) % 8
                            for gg in range(2):
                                vsrc = v_ag[hh][2 * half + gg].rearrange(
                                    "(t p) d -> p t d", p=128
                                )
                                nc.sync.dma_start(
                                    vf[:, :, gg * 512 : (gg + 1) * 512],
                                    vsrc[:, t0 : t0 + 4, :],
                                )
                            for kj in range(4):
                                kt = 4 * kg + kj
                                for sub in range(2):
                                    nc.tensor.matmul(
                                        po[:, sub * 512 : (sub + 1) * 512],
                                        pt[:, kt, :],
                                        vf[:, kj, sub * 512 : (sub + 1) * 512],
                                        start=(kt == 0),
                                        stop=(kt == nkt - 1),
                                    )
                        y_sb = ysbp.tile([128, 1024], F32, tag="ysb")
                        if half == 0:
                            nc.scalar.copy(out=y_sb[:], in_=po[:])
                        else:
                            nc.vector.tensor_copy(out=y_sb[:], in_=po[:])
                        nc.sync.dma_start(
                            out[s * 128 : (s + 1) * 128, c0 : c0 + 1024],
                            y_sb[:],
                        )

                issue_slab_C(3)
                issue_slab_C(2)
                issue_slab_C(1)
                issue_slab_C(0)
                ep.release()
                issue_slab_D(3)
                issue_slab_D(2)
                issue_slab_D(1)
                issue_slab_D(0)
        qkp.release()

    nc.compile()
    return nc


_NC_CACHE = None


def kernel(x, W_q, W_k, W_v, W_o):
    global _NC_CACHE
    x = np.asarray(x, dtype=np.float32)
    W_q = np.asarray(W_q, dtype=np.float32)
    W_k = np.asarray(W_k, dtype=np.float32)
    W_v = np.asarray(W_v, dtype=np.float32)
    W_o = np.asarray(W_o, dtype=np.float32)
    if _NC_CACHE is None:
        _NC_CACHE = build()
    nc = _NC_CACHE

    Wvo = W_v @ W_o  # fp32 host-side fold
    xT = [np.ascontiguousarray(x[g].T) for g in range(2)]
    xT_bf = [t.astype(ml_dtypes.bfloat16) for t in xT]
    in_maps = []
    for c in range(8):
        g, r = divmod(c, 4)
        in_maps.append(
            {
                "x": xT[g],
                "xbf": xT_bf[g],
                "wq": np.ascontiguousarray(W_q[:, 512 * r : 512 * (r + 1)]),
                "wk": np.ascontiguousarray(W_k[:, 512 * r : 512 * (r + 1)]),
                "wv": np.ascontiguousarray(Wvo[:, 512 * r : 512 * (r + 1)]).astype(ml_dtypes.bfloat16),
            }
        )
    res = run_bass_kernel_spmd(nc, in_maps, core_ids=list(range(8)))
    Y = np.empty((2, S, D), dtype=np.float32)
    for c in range(8):
        g, r = divmod(c, 4)
        o = res.results[c]["out"]
        for s_idx in range(4):
            t = 4 * s_idx + r
            Y[g, t * 128 : (t + 1) * 128, :] = o[s_idx * 128 : (s_idx + 1) * 128, :]
    return Y


# revision 3
# speedup vs baseline: 1.1959x; 1.0233x over previous
"""Distributed attention kernel for 8 trn2 NeuronCores (v3).

Reference semantics (B=2, S=2048, D=2048, H=16, dh=128):
  q = x@W_q, k = x@W_k  (per-head split), v = x@W_v (full width)
  scores = q@k^T per head; (scores + triu(-1e9)) * 1/sqrt(dh); softmax
  out = (sum_h probs_h) @ v @ W_o        <- heads summed, v full width

Algebraic fold: out = P @ (x @ (W_v @ W_o)) = P @ U with U = x @ Wvo
precomputed host-side (fp32) — no final W_o matmul phase.

Sharding: 2 groups of 4 cores (batch parallel); within a group, rank r
owns heads {4r..4r+3} (cols of W_q/W_k), cols [512r, 512r+512) of Wvo.

Schedule: projection quarter qd (q/k/U for q-rows [512qd,512qd+512))
is interleaved at the pass level with the C tiles of slab qd-1
(scores exact-causal-trimmed + softmax), so the PE always has dense
matmul work while ACT/DVE run softmax.  P = sum_h probs_h is computed
on DVE (scalar_tensor_tensor with 1/Z per-partition scalars) — no PE
work.  U quarters AllGather per quarter; P slabs ReduceScatter per
slab right after their last tile.  D phases (P^T transpose + P@U into
the output) run at the end, overlapping the tail of the RS chain.
"""

import math

import numpy as np
import ml_dtypes

import concourse.bass as bass
import concourse.mybir as mybir
import concourse.tile as tile
from concourse import bacc
from concourse.bass_utils import run_bass_kernel_spmd
from concourse.masks import make_identity

F32 = mybir.dt.float32
F32R = mybir.dt.float32r
BF16 = mybir.dt.bfloat16

S = 2048
D = 2048
DH = 128
NT = S // 128  # 16 q/k tiles
SCALE = 1.0 / math.sqrt(DH)
GROUPS = [[0, 1, 2, 3], [4, 5, 6, 7]]
NEG = -1e9


def build():
    nc = bacc.Bacc("TRN2", target_bir_lowering=False, debug=False, num_devices=8)

    x = nc.declare_dram_parameter("x", [D, S], F32R, isOutput=False)  # x^T
    xbf = nc.declare_dram_parameter("xbf", [D, S], BF16, isOutput=False)
    wq = nc.declare_dram_parameter("wq", [D, 512], F32R, isOutput=False)
    wk = nc.declare_dram_parameter("wk", [D, 512], F32R, isOutput=False)
    wv = nc.declare_dram_parameter("wv", [D, 512], BF16, isOutput=False)  # Wvo slice
    out = nc.declare_dram_parameter("out", [512, D], F32, isOutput=True)

    v_local = nc.dram_tensor("v_local", [S, 512], BF16)
    v_ag = [nc.dram_tensor(f"v_ag{h}", [4, 512, 512], BF16) for h in range(4)]
    p_part = [nc.dram_tensor(f"p_part{s}", [512, 512 * (s + 1)], BF16) for s in range(4)]
    p_recv = [nc.dram_tensor(f"p_recv{s}", [128, 512 * (s + 1)], BF16) for s in range(4)]

    with tile.TileContext(nc) as tc:
        qkp = tc.alloc_tile_pool(name="qk", bufs=1)
        kT = qkp.tile([128, 4, S], F32R)  # [dh-part, head, k-pos], persistent
        with tc.tile_pool(name="const", bufs=1) as cst:
            ident = cst.tile([128, 128], F32)
            make_identity(nc, ident)
            ident_bf = cst.tile([128, 128], BF16)
            nc.vector.tensor_copy(out=ident_bf[:], in_=ident[:])
            # diagonal-tile mask: 0 where col <= row else -1e9
            mask128 = cst.tile([128, 128], BF16)
            nc.gpsimd.memset(mask128[:], 0.0)
            nc.gpsimd.affine_select(
                out=mask128[:],
                in_=mask128[:],
                compare_op=mybir.AluOpType.is_ge,
                fill=NEG,
                base=0,
                pattern=[[-1, 128]],
                channel_multiplier=1,
            )

            wq_src = wq.rearrange("(t p) c -> p t c", p=128)
            wk_src = wk.rearrange("(t p) c -> p t c", p=128)
            wv_src = wv.rearrange("(t p) c -> p t c", p=128)
            x_src = x.rearrange("(t p) s -> p t s", p=128)
            xbf_src = xbf.rearrange("(t p) s -> p t s", p=128)

            # ---------- Region 1: projections interleaved with C ----------
            with (
                tc.tile_pool(name="wsb", bufs=1) as wsb,
                tc.tile_pool(name="qTp", bufs=2) as qTp,
                tc.tile_pool(name="xq_pool", bufs=1) as xqp,
                tc.tile_pool(name="xbf_pool", bufs=4) as xbp,
                tc.tile_pool(name="drain", bufs=4) as drp,
                tc.tile_pool(name="e1p", bufs=2) as e1p,
                tc.tile_pool(name="pacc", bufs=4) as pap,
                tc.tile_pool(name="small", bufs=48) as smp,
                tc.tile_pool(name="pj_ps", bufs=4, space="PSUM") as pjp,
                tc.tile_pool(name="sc_ps", bufs=2, space="PSUM") as scp,
            ):
                wq_sb = wsb.tile([128, NT, 512], F32R)
                wk_sb = wsb.tile([128, NT, 512], F32R)
                wv_sb = wsb.tile([128, NT, 512], BF16)
                nc.sync.dma_start(wq_sb[:, 0:4, :], wq_src[:, 0:4, :])

                def proj_pass(dst_tile, dst_off, wsrc, xq, eng):
                    """q or k projection pass for one quarter."""
                    psums = [
                        pjp.tile([128, 512], F32, tag="ps", name=f"pj{_j}")
                        for _j in range(4)
                    ]
                    for Dt in range(NT):
                        for dt in range(4):
                            nc.tensor.matmul(
                                psums[dt][:],
                                wsrc[:, Dt, dt * 128 : (dt + 1) * 128],
                                xq[:, Dt, :],
                                start=(Dt == 0),
                                stop=(Dt == NT - 1),
                            )
                    for dt in range(4):
                        if eng == "s":
                            nc.scalar.copy(
                                out=dst_tile[:, dt, dst_off : dst_off + 512],
                                in_=psums[dt][:],
                            )
                        else:
                            nc.vector.tensor_copy(
                                out=dst_tile[:, dt, dst_off : dst_off + 512],
                                in_=psums[dt][:],
                            )

                def u_pass(qd):
                    s0 = qd * 512
                    psums = [
                        pjp.tile([128, 512], F32, tag="ps", name=f"pv{_j}")
                        for _j in range(4)
                    ]
                    for g4 in range(4):
                        xb_t = xbp.tile([128, 4, 512], BF16, tag="xb")
                        nc.sync.dma_start(
                            xb_t[:], xbf_src[:, 4 * g4 : 4 * g4 + 4, s0 : s0 + 512]
                        )
                        for dj in range(4):
                            Dt = 4 * g4 + dj
                            for sb in range(4):
                                nc.tensor.matmul(
                                    psums[sb][:],
                                    xb_t[:, dj, sb * 128 : (sb + 1) * 128],
                                    wv_sb[:, Dt, :],
                                    start=(Dt == 0),
                                    stop=(Dt == NT - 1),
                                )
                    for sb in range(4):
                        v_sb = drp.tile([128, 512], BF16, tag="vsb")
                        nc.vector.tensor_copy(out=v_sb[:], in_=psums[sb][:])
                        r0 = s0 + sb * 128
                        nc.sync.dma_start(v_local[r0 : r0 + 128, :], v_sb[:])

                def c_tile(i, qT_prev):
                    """Scores + softmax + P (on DVE) for q-tile i."""
                    s = i // 4
                    m = i % 4
                    vw = 128 * (i + 1)          # exact causal width
                    kw = 512 * (s + 1)          # slab-padded width
                    c512 = (vw + 511) // 512
                    ntile = (vw + 1023) // 1024
                    # scores for 4 heads into [128,1024] psum tiles
                    sts = []
                    for h in range(4):
                        st = [
                            scp.tile([128, 1024], F32, tag="S", name=f"sc{i}h{h}t{_t}")
                            for _t in range(ntile)
                        ]
                        for kc in range(c512):
                            w = min(512, vw - 512 * kc)
                            diag = kc == c512 - 1
                            tgt = st[kc // 2][:, (kc % 2) * 512 : (kc % 2) * 512 + w]
                            nc.tensor.matmul(
                                tgt,
                                qT_prev[:, h, m * 128 : (m + 1) * 128],
                                kT[:, h, kc * 512 : kc * 512 + w],
                                start=True,
                                stop=not diag,
                            )
                            if diag:
                                nc.tensor.matmul(
                                    tgt[:, w - 128 : w],
                                    ident_bf[:],
                                    mask128[:],
                                    start=False,
                                    stop=True,
                                )
                        sts.append(st)
                    # softmax + P accumulation (ping-pong bf16 P_acc on DVE)
                    p_cur = None
                    for h in range(4):
                        st = sts[h]
                        mx = None
                        for t in range(ntile):
                            w = min(vw - 1024 * t, 1024)
                            mxt = smp.tile([128, 1], F32, tag="mx")
                            nc.vector.reduce_max(
                                out=mxt[:], in_=st[t][:, :w],
                                axis=mybir.AxisListType.X,
                            )
                            if mx is None:
                                mx = mxt
                            else:
                                mxn = smp.tile([128, 1], F32, tag="mx")
                                nc.vector.tensor_tensor(
                                    out=mxn[:], in0=mx[:], in1=mxt[:],
                                    op=mybir.AluOpType.max,
                                )
                                mx = mxn
                        nmS = smp.tile([128, 1], F32, tag="mx")
                        nc.vector.tensor_scalar_mul(nmS[:], mx[:], -SCALE)
                        e1 = e1p.tile([128, 2048], BF16, tag="E", name=f"e{i}h{h}")
                        rzs = []
                        for t in range(ntile):
                            w = min(vw - 1024 * t, 1024)
                            rz = smp.tile([128, 1], F32, tag="mx", name=f"rz{t}")
                            nc.scalar.activation(
                                out=e1[:, 1024 * t : 1024 * t + w],
                                in_=st[t][:, :w],
                                func=mybir.ActivationFunctionType.Exp,
                                bias=nmS[:],
                                scale=SCALE,
                                accum_out=rz[:],
                            )
                            rzs.append(rz)
                        if ntile == 2:
                            zt = smp.tile([128, 1], F32, tag="mx")
                            nc.vector.tensor_tensor(
                                out=zt[:], in0=rzs[0][:], in1=rzs[1][:],
                                op=mybir.AluOpType.add,
                            )
                        else:
                            zt = rzs[0]
                        ri = smp.tile([128, 1], F32, tag="mx")
                        nc.vector.reciprocal(out=ri[:], in_=zt[:])
                        p_new = pap.tile([128, 2048], BF16, tag="PA", name=f"pa{i}h{h}")
                        if p_cur is None:
                            nc.vector.tensor_scalar_mul(
                                p_new[:, :vw], e1[:, :vw], ri[:]
                            )
                        else:
                            nc.vector.scalar_tensor_tensor(
                                out=p_new[:, :vw],
                                in0=e1[:, :vw],
                                scalar=ri[:],
                                in1=p_cur[:, :vw],
                                op0=mybir.AluOpType.mult,
                                op1=mybir.AluOpType.add,
                            )
                        p_cur = p_new
                    if kw > vw:
                        nc.vector.memset(p_cur[:, vw:kw], 0.0)
                    nc.sync.dma_start(
                        p_part[s][m * 128 : (m + 1) * 128, :],
                        p_cur[:, :kw],
                    )

                def issue_rs(s):
                    nc.gpsimd.collective_compute(
                        "ReduceScatter",
                        mybir.AluOpType.add,
                        ins=[p_part[s][:]],
                        outs=[p_recv[s][:]],
                        replica_groups=GROUPS,
                    )

                def issue_ag(qd):
                    nc.gpsimd.collective_compute(
                        "AllGather",
                        mybir.AluOpType.bypass,
                        ins=[v_local[qd * 512 : (qd + 1) * 512, :]],
                        outs=[v_ag[qd][:]],
                        replica_groups=GROUPS,
                    )

                qT_prev = None
                for qd in range(4):
                    s0 = qd * 512
                    xq = xqp.tile([128, NT, 512], F32R, tag="xq")
                    for g4 in range(4):
                        nc.sync.dma_start(
                            xq[:, 4 * g4 : 4 * g4 + 4, :],
                            x_src[:, 4 * g4 : 4 * g4 + 4, s0 : s0 + 512],
                        )
                        if qd == 0 and g4 >= 1:
                            nc.sync.dma_start(
                                wq_sb[:, 4 * g4 : 4 * g4 + 4, :],
                                wq_src[:, 4 * g4 : 4 * g4 + 4, :],
                            )
                    if qd == 0:
                        for g4 in range(4):
                            nc.sync.dma_start(
                                wk_sb[:, 4 * g4 : 4 * g4 + 4, :],
                                wk_src[:, 4 * g4 : 4 * g4 + 4, :],
                            )
                        for g4 in range(4):
                            nc.sync.dma_start(
                                wv_sb[:, 4 * g4 : 4 * g4 + 4, :],
                                wv_src[:, 4 * g4 : 4 * g4 + 4, :],
                            )
                    qT = qTp.tile([128, 4, 512], F32R, tag="qT", name=f"qT{qd}")
                    ci = 4 * (qd - 1)  # C tiles of previous quarter's slab
                    proj_pass(qT, 0, wq_sb, xq, "s")
                    if qd >= 1:
                        c_tile(ci + 0, qT_prev)
                    proj_pass(kT, s0, wk_sb, xq, "v")
                    if qd >= 1:
                        c_tile(ci + 1, qT_prev)
                    u_pass(qd)
                    if qd >= 1:
                        c_tile(ci + 2, qT_prev)
                        c_tile(ci + 3, qT_prev)
                    issue_ag(qd)
                    if qd >= 1:
                        issue_rs(qd - 1)
                    qT_prev = qT
                for j in range(4):
                    c_tile(12 + j, qT_prev)
                issue_rs(3)

            # ---------------- Region 2: D phases ----------------
            with (
                tc.tile_pool(name="rp", bufs=2) as rp,
                tc.tile_pool(name="ptp", bufs=2) as ptp,
                tc.tile_pool(name="vfp", bufs=2) as vfp,
                tc.tile_pool(name="ysb", bufs=2) as ysbp,
                tc.tile_pool(name="dpo", bufs=2, space="PSUM") as dpo,
                tc.tile_pool(name="tr_ps", bufs=2, space="PSUM") as trp,
            ):
                def issue_slab_D(s):
                    kw = 512 * (s + 1)
                    nkt = 4 * (s + 1)
                    pown = rp.tile([128, 2048], BF16, tag="POW")
                    nc.sync.dma_start(pown[:, :kw], p_recv[s][:])
                    pt = ptp.tile([128, NT, 128], BF16, tag="PT")
                    for kg in range((nkt + 7) // 8):
                        nsl = min(nkt - 8 * kg, 8)
                        tr = trp.tile([128, 8, 128], BF16, tag="TR", name=f"tr{s}_{kg}")
                        for j in range(nsl):
                            kt = 8 * kg + j
                            nc.tensor.transpose(
                                tr[:, j, :],
                                pown[:, kt * 128 : (kt + 1) * 128],
                                ident_bf[:],
                            )
                        nc.vector.tensor_copy(
                            out=pt[:, 8 * kg : 8 * kg + nsl, :], in_=tr[:, :nsl, :]
                        )
                    for half in range(2):
                        c0 = half * 1024
                        po = dpo.tile([128, 1024], F32, tag="PO", name=f"po{s}_{half}")
                        for kg in range(s + 1):
                            vf = vfp.tile([128, 4, 1024], BF16, tag="VF")
                            for gg in range(2):
                                vsrc = v_ag[kg][2 * half + gg].rearrange(
                                    "(t p) d -> p t d", p=128
                                )
                                nc.sync.dma_start(
                                    vf[:, :, gg * 512 : (gg + 1) * 512],
                                    vsrc[:, 0:4, :],
                                )
                            for kj in range(4):
                                kt = 4 * kg + kj
                                for sub in range(2):
                                    nc.tensor.matmul(
                                        po[:, sub * 512 : (sub + 1) * 512],
                                        pt[:, kt, :],
                                        vf[:, kj, sub * 512 : (sub + 1) * 512],
                                        start=(kt == 0),
                                        stop=(kt == nkt - 1),
                                    )
                        y_sb = ysbp.tile([128, 1024], F32, tag="ysb")
                        if half == 0:
                            nc.scalar.copy(out=y_sb[:], in_=po[:])
                        else:
                            nc.vector.tensor_copy(out=y_sb[:], in_=po[:])
                        nc.sync.dma_start(
                            out[s * 128 : (s + 1) * 128, c0 : c0 + 1024],
                            y_sb[:],
                        )

                issue_slab_D(0)
                issue_slab_D(1)
                issue_slab_D(2)
                issue_slab_D(3)
        qkp.release()

    nc.compile()
    return nc


_NC_CACHE = None


def kernel(x, W_q, W_k, W_v, W_o):
    global _NC_CACHE
    x = np.asarray(x, dtype=np.float32)
    W_q = np.asarray(W_q, dtype=np.float32)
    W_k = np.asarray(W_k, dtype=np.float32)
    W_v = np.asarray(W_v, dtype=np.float32)
    W_o = np.asarray(W_o, dtype=np.float32)
    if _NC_CACHE is None:
        _NC_CACHE = build()
    nc = _NC_CACHE

    Wvo = W_v @ W_o  # fp32 host-side fold
    xT = [np.ascontiguousarray(x[g].T) for g in range(2)]
    xT_bf = [t.astype(ml_dtypes.bfloat16) for t in xT]
    in_maps = []
    for c in range(8):
        g, r = divmod(c, 4)
        in_maps.append(
            {
                "x": xT[g],
                "xbf": xT_bf[g],
                "wq": np.ascontiguousarray(W_q[:, 512 * r : 512 * (r + 1)]),
                "wk": np.ascontiguousarray(W_k[:, 512 * r : 512 * (r + 1)]),
                "wv": np.ascontiguousarray(Wvo[:, 512 * r : 512 * (r + 1)]).astype(ml_dtypes.bfloat16),
            }
        )
    res = run_bass_kernel_spmd(nc, in_maps, core_ids=list(range(8)))
    Y = np.empty((2, S, D), dtype=np.float32)
    for c in range(8):
        g, r = divmod(c, 4)
        o = res.results[c]["out"]
        for s_idx in range(4):
            t = 4 * s_idx + r
            Y[g, t * 128 : (t + 1) * 128, :] = o[s_idx * 128 : (s_idx + 1) * 128, :]
    return Y


# revision 4
# speedup vs baseline: 1.2085x; 1.0106x over previous
"""Distributed attention kernel for 8 trn2 NeuronCores (v3).

Reference semantics (B=2, S=2048, D=2048, H=16, dh=128):
  q = x@W_q, k = x@W_k  (per-head split), v = x@W_v (full width)
  scores = q@k^T per head; (scores + triu(-1e9)) * 1/sqrt(dh); softmax
  out = (sum_h probs_h) @ v @ W_o        <- heads summed, v full width

Algebraic fold: out = P @ (x @ (W_v @ W_o)) = P @ U with U = x @ Wvo
precomputed host-side (fp32) — no final W_o matmul phase.

Sharding: 2 groups of 4 cores (batch parallel); within a group, rank r
owns heads {4r..4r+3} (cols of W_q/W_k), cols [512r, 512r+512) of Wvo.

Schedule: per q-row quarter qd, projection passes run q, U, k (each a
single-psum 16-matmul stream), woven one-pass-per-head-piece with the
C pieces (scores exact-causal-trimmed + softmax + DVE P-accumulate) of
slab qd-1, so the PE never idles on softmax latency.  U AllGathers in
3 chunks as rows complete; P slabs ReduceScatter right after their
last tile.  D loads the full gathered U into SBUF once (8MB), then
computes P^T (PE transpose) and OUT = P@U per slab straight into the
output tensor.
"""

import math

import numpy as np
import ml_dtypes

import concourse.bass as bass
import concourse.mybir as mybir
import concourse.tile as tile
from concourse import bacc
from concourse.bass_utils import run_bass_kernel_spmd
from concourse.masks import make_identity

F32 = mybir.dt.float32
F32R = mybir.dt.float32r
BF16 = mybir.dt.bfloat16

S = 2048
D = 2048
DH = 128
NT = S // 128  # 16 q/k tiles
SCALE = 1.0 / math.sqrt(DH)
GROUPS = [[0, 1, 2, 3], [4, 5, 6, 7]]
NEG = -1e9


def build():
    nc = bacc.Bacc("TRN2", target_bir_lowering=False, debug=False, num_devices=8)

    x = nc.declare_dram_parameter("x", [D, S], F32R, isOutput=False)  # x^T
    xbf = nc.declare_dram_parameter("xbf", [D, S], BF16, isOutput=False)
    wq = nc.declare_dram_parameter("wq", [D, 512], F32R, isOutput=False)
    wk = nc.declare_dram_parameter("wk", [D, 512], F32R, isOutput=False)
    wv = nc.declare_dram_parameter("wv", [D, 512], BF16, isOutput=False)  # Wvo slice
    out = nc.declare_dram_parameter("out", [512, D], F32, isOutput=True)

    v_local = nc.dram_tensor("v_local", [S, 512], BF16)
    # U AllGather chunks: rows [0,1024), [1024,1536), [1536,2048)
    v_ag = [
        nc.dram_tensor("v_ag0", [4, 1024, 512], BF16),
        nc.dram_tensor("v_ag2", [4, 512, 512], BF16),
        nc.dram_tensor("v_ag3", [4, 512, 512], BF16),
    ]
    p_part = [nc.dram_tensor(f"p_part{s}", [512, 512 * (s + 1)], BF16) for s in range(4)]
    p_recv = [nc.dram_tensor(f"p_recv{s}", [128, 512 * (s + 1)], BF16) for s in range(4)]

    with tile.TileContext(nc) as tc:
        qkp = tc.alloc_tile_pool(name="qk", bufs=1)
        kT = qkp.tile([128, 4, S], F32R)  # [dh-part, head, k-pos]
        with tc.tile_pool(name="const", bufs=1) as cst:
            ident = cst.tile([128, 128], F32)
            make_identity(nc, ident)
            ident_bf = cst.tile([128, 128], BF16)
            nc.vector.tensor_copy(out=ident_bf[:], in_=ident[:])
            mask128 = cst.tile([128, 128], BF16)
            nc.gpsimd.memset(mask128[:], 0.0)
            nc.gpsimd.affine_select(
                out=mask128[:],
                in_=mask128[:],
                compare_op=mybir.AluOpType.is_ge,
                fill=NEG,
                base=0,
                pattern=[[-1, 128]],
                channel_multiplier=1,
            )

            wq_src = wq.rearrange("(t p) c -> p t c", p=128)
            wk_src = wk.rearrange("(t p) c -> p t c", p=128)
            wv_src = wv.rearrange("(t p) c -> p t c", p=128)
            x_src = x.rearrange("(t p) s -> p t s", p=128)
            xbf_src = xbf.rearrange("(t p) s -> p t s", p=128)

            # ---------- Region 1: projections woven with C ----------
            with (
                tc.tile_pool(name="wsb", bufs=1) as wsb,
                tc.tile_pool(name="qTp", bufs=2) as qTp,
                tc.tile_pool(name="xq_pool", bufs=1) as xqp,
                tc.tile_pool(name="xbf_pool", bufs=1) as xbp,
                tc.tile_pool(name="drain", bufs=2) as drp,
                tc.tile_pool(name="e1p", bufs=2) as e1p,
                tc.tile_pool(name="pacc", bufs=3) as pap,
                tc.tile_pool(name="small", bufs=48) as smp,
                tc.tile_pool(name="pj_ps", bufs=2, space="PSUM") as pjp,
                tc.tile_pool(name="sc_ps", bufs=3, space="PSUM") as scp,
            ):
                wq_sb = wsb.tile([128, NT, 512], F32R)
                wk_sb = wsb.tile([128, NT, 512], F32R)
                wv_sb = wsb.tile([128, NT, 512], BF16)
                nc.sync.dma_start(wq_sb[:, 0:4, :], wq_src[:, 0:4, :])

                def qk_pass_dt(dst_tile, dst_off, wsrc, xq, dt, eng):
                    ps = pjp.tile([128, 512], F32, tag="ps", name=f"pj{dt}")
                    for Dt in range(NT):
                        nc.tensor.matmul(
                            ps[:],
                            wsrc[:, Dt, dt * 128 : (dt + 1) * 128],
                            xq[:, Dt, :],
                            start=(Dt == 0),
                            stop=(Dt == NT - 1),
                        )
                    if eng == "s":
                        nc.scalar.copy(
                            out=dst_tile[:, dt, dst_off : dst_off + 512], in_=ps[:]
                        )
                    else:
                        nc.vector.tensor_copy(
                            out=dst_tile[:, dt, dst_off : dst_off + 512], in_=ps[:]
                        )

                def u_pass_sb(qd, xb, sb):
                    s0 = qd * 512
                    ps = pjp.tile([128, 512], F32, tag="ps", name=f"pu{sb}")
                    for Dt in range(NT):
                        nc.tensor.matmul(
                            ps[:],
                            xb[:, Dt, sb * 128 : (sb + 1) * 128],
                            wv_sb[:, Dt, :],
                            start=(Dt == 0),
                            stop=(Dt == NT - 1),
                        )
                    v_sb = drp.tile([128, 512], BF16, tag="vsb")
                    nc.vector.tensor_copy(out=v_sb[:], in_=ps[:])
                    r0 = s0 + sb * 128
                    nc.sync.dma_start(v_local[r0 : r0 + 128, :], v_sb[:])

                def c_piece(i, h, qT_prev, cell):
                    """Scores + softmax + P-accumulate for (q-tile i, head h)."""
                    s = i // 4
                    m = i % 4
                    vw = 128 * (i + 1)
                    kw = 512 * (s + 1)
                    c512 = (vw + 511) // 512
                    ntile = (vw + 1023) // 1024
                    st = [
                        scp.tile([128, 1024], F32, tag="S", name=f"sc{i}h{h}t{_t}")
                        for _t in range(ntile)
                    ]
                    for kc in range(c512):
                        w = min(512, vw - 512 * kc)
                        diag = kc == c512 - 1
                        tgt = st[kc // 2][:, (kc % 2) * 512 : (kc % 2) * 512 + w]
                        nc.tensor.matmul(
                            tgt,
                            qT_prev[:, h, m * 128 : (m + 1) * 128],
                            kT[:, h, kc * 512 : kc * 512 + w],
                            start=True,
                            stop=not diag,
                        )
                        if diag:
                            nc.tensor.matmul(
                                tgt[:, w - 128 : w],
                                ident_bf[:],
                                mask128[:],
                                start=False,
                                stop=True,
                            )
                    mx = None
                    for t in range(ntile):
                        w = min(vw - 1024 * t, 1024)
                        mxt = smp.tile([128, 1], F32, tag="mx")
                        nc.vector.reduce_max(
                            out=mxt[:], in_=st[t][:, :w], axis=mybir.AxisListType.X
                        )
                        if mx is None:
                            mx = mxt
                        else:
                            mxn = smp.tile([128, 1], F32, tag="mx")
                            nc.vector.tensor_tensor(
                                out=mxn[:], in0=mx[:], in1=mxt[:],
                                op=mybir.AluOpType.max,
                            )
                            mx = mxn
                    nmS = smp.tile([128, 1], F32, tag="mx")
                    nc.vector.tensor_scalar_mul(nmS[:], mx[:], -SCALE)
                    e1 = e1p.tile([128, 2048], BF16, tag="E", name=f"e{i}h{h}")
                    rzs = []
                    for t in range(ntile):
                        w = min(vw - 1024 * t, 1024)
                        rz = smp.tile([128, 1], F32, tag="mx", name=f"rz{t}")
                        nc.scalar.activation(
                            out=e1[:, 1024 * t : 1024 * t + w],
                            in_=st[t][:, :w],
                            func=mybir.ActivationFunctionType.Exp,
                            bias=nmS[:],
                            scale=SCALE,
                            accum_out=rz[:],
                        )
                        rzs.append(rz)
                    if ntile == 2:
                        zt = smp.tile([128, 1], F32, tag="mx")
                        nc.vector.tensor_tensor(
                            out=zt[:], in0=rzs[0][:], in1=rzs[1][:],
                            op=mybir.AluOpType.add,
                        )
                    else:
                        zt = rzs[0]
                    ri = smp.tile([128, 1], F32, tag="mx")
                    nc.vector.reciprocal(out=ri[:], in_=zt[:])
                    p_new = pap.tile([128, 2048], BF16, tag="PA", name=f"pa{i}h{h}")
                    if cell[0] is None:
                        nc.vector.tensor_scalar_mul(p_new[:, :vw], e1[:, :vw], ri[:])
                    else:
                        nc.vector.scalar_tensor_tensor(
                            out=p_new[:, :vw],
                            in0=e1[:, :vw],
                            scalar=ri[:],
                            in1=cell[0][:, :vw],
                            op0=mybir.AluOpType.mult,
                            op1=mybir.AluOpType.add,
                        )
                    cell[0] = p_new
                    if h == 3:
                        if kw > vw:
                            nc.vector.memset(p_new[:, vw:kw], 0.0)
                        nc.sync.dma_start(
                            p_part[s][m * 128 : (m + 1) * 128, :],
                            p_new[:, :kw],
                        )
                        cell[0] = None

                def issue_rs(s):
                    nc.gpsimd.collective_compute(
                        "ReduceScatter",
                        mybir.AluOpType.add,
                        ins=[p_part[s][:]],
                        outs=[p_recv[s][:]],
                        replica_groups=GROUPS,
                    )

                def issue_ag(idx, r0, rows):
                    nc.gpsimd.collective_compute(
                        "AllGather",
                        mybir.AluOpType.bypass,
                        ins=[v_local[r0 : r0 + rows, :]],
                        outs=[v_ag[idx][:]],
                        replica_groups=GROUPS,
                    )

                qT_prev = None
                for qd in range(4):
                    s0 = qd * 512
                    xq = xqp.tile([128, NT, 512], F32R, tag="xq")
                    xb = xbp.tile([128, NT, 512], BF16, tag="xb")
                    for g4 in range(4):
                        nc.sync.dma_start(
                            xq[:, 4 * g4 : 4 * g4 + 4, :],
                            x_src[:, 4 * g4 : 4 * g4 + 4, s0 : s0 + 512],
                        )
                        if qd == 0 and g4 >= 1:
                            nc.sync.dma_start(
                                wq_sb[:, 4 * g4 : 4 * g4 + 4, :],
                                wq_src[:, 4 * g4 : 4 * g4 + 4, :],
                            )
                    for g4 in range(4):
                        nc.sync.dma_start(
                            xb[:, 4 * g4 : 4 * g4 + 4, :],
                            xbf_src[:, 4 * g4 : 4 * g4 + 4, s0 : s0 + 512],
                        )
                    if qd == 0:
                        for g4 in range(4):
                            nc.sync.dma_start(
                                wk_sb[:, 4 * g4 : 4 * g4 + 4, :],
                                wk_src[:, 4 * g4 : 4 * g4 + 4, :],
                            )
                        for g4 in range(4):
                            nc.sync.dma_start(
                                wv_sb[:, 4 * g4 : 4 * g4 + 4, :],
                                wv_src[:, 4 * g4 : 4 * g4 + 4, :],
                            )
                    qT = qTp.tile([128, 4, 512], F32R, tag="qT", name=f"qT{qd}")
                    # pass order: q, U, k  (C pieces only need kT <= qd-1)
                    passes = []
                    for dt in range(4):
                        passes.append(
                            lambda dt=dt: qk_pass_dt(qT, 0, wq_sb, xq, dt, "s")
                        )
                    for sb in range(4):
                        passes.append(lambda sb=sb: u_pass_sb(qd, xb, sb))
                    for dt in range(4):
                        passes.append(
                            lambda dt=dt: qk_pass_dt(kT, s0, wk_sb, xq, dt, "v")
                        )
                    # C pieces of slab qd-1
                    pieces = []
                    if qd >= 1:
                        cell = [None]
                        for i in range(4 * (qd - 1), 4 * (qd - 1) + 4):
                            for h in range(4):
                                pieces.append(
                                    lambda i=i, h=h, q=qT_prev, c=cell: c_piece(
                                        i, h, q, c
                                    )
                                )
                    done = 0
                    for j, p in enumerate(passes):
                        p()
                        want = (j + 1) * len(pieces) // len(passes)
                        while done < want:
                            pieces[done]()
                            done += 1
                    while done < len(pieces):
                        pieces[done]()
                        done += 1
                    if qd == 1:
                        issue_ag(0, 0, 1024)
                    elif qd == 2:
                        issue_ag(1, 1024, 512)
                    elif qd == 3:
                        issue_ag(2, 1536, 512)
                    if qd >= 1:
                        issue_rs(qd - 1)
                    qT_prev = qT
                cell = [None]
                for i in range(12, 16):
                    for h in range(4):
                        c_piece(i, h, qT_prev, cell)
                issue_rs(3)

            # ---------------- Region 2: D phases ----------------
            with (
                tc.tile_pool(name="usb", bufs=1) as usbp,
                tc.tile_pool(name="rp", bufs=2) as rp,
                tc.tile_pool(name="ptp", bufs=2) as ptp,
                tc.tile_pool(name="ysb", bufs=2) as ysbp,
                tc.tile_pool(name="d_ps", bufs=1, space="PSUM") as dpo,
                tc.tile_pool(name="tr_ps", bufs=2, space="PSUM") as trp,
            ):
                usb = usbp.tile([128, NT, 2048], BF16)  # full U, [k-part, kt, dv]
                for kg in range(4):
                    if kg < 2:
                        src_t, row0 = 0, kg * 512
                    else:
                        src_t, row0 = kg - 1, 0
                    for gg in range(4):
                        vsrc = v_ag[src_t][gg].rearrange("(t p) d -> p t d", p=128)
                        nc.sync.dma_start(
                            usb[:, 4 * kg : 4 * kg + 4, gg * 512 : (gg + 1) * 512],
                            vsrc[:, row0 // 128 : row0 // 128 + 4, :],
                        )

                def issue_slab_D(s):
                    kw = 512 * (s + 1)
                    nkt = 4 * (s + 1)
                    pown = rp.tile([128, 2048], BF16, tag="POW")
                    nc.scalar.dma_start(pown[:, :kw], p_recv[s][:])
                    pt = ptp.tile([128, NT, 128], BF16, tag="PT")
                    for kg in range((nkt + 7) // 8):
                        nsl = min(nkt - 8 * kg, 8)
                        tr = trp.tile([128, 8, 128], BF16, tag="TR", name=f"tr{s}_{kg}")
                        for j in range(nsl):
                            kt = 8 * kg + j
                            nc.tensor.transpose(
                                tr[:, j, :],
                                pown[:, kt * 128 : (kt + 1) * 128],
                                ident_bf[:],
                            )
                        nc.vector.tensor_copy(
                            out=pt[:, 8 * kg : 8 * kg + nsl, :], in_=tr[:, :nsl, :]
                        )
                    po = dpo.tile([128, 2048], F32, tag="PO", name=f"po{s}")
                    for kt in range(nkt):
                        for quad in range(4):
                            nc.tensor.matmul(
                                po[:, quad * 512 : (quad + 1) * 512],
                                pt[:, kt, :],
                                usb[:, kt, quad * 512 : (quad + 1) * 512],
                                start=(kt == 0),
                                stop=(kt == nkt - 1),
                            )
                    for half in range(2):
                        y_sb = ysbp.tile([128, 1024], F32, tag="ysb")
                        if half == 0:
                            nc.scalar.copy(out=y_sb[:], in_=po[:, 0:1024])
                        else:
                            nc.vector.tensor_copy(out=y_sb[:], in_=po[:, 1024:2048])
                        nc.sync.dma_start(
                            out[s * 128 : (s + 1) * 128, half * 1024 : half * 1024 + 1024],
                            y_sb[:],
                        )

                issue_slab_D(0)
                issue_slab_D(1)
                issue_slab_D(2)
                issue_slab_D(3)
        qkp.release()

    nc.compile()
    return nc


_NC_CACHE = None


def kernel(x, W_q, W_k, W_v, W_o):
    global _NC_CACHE
    x = np.asarray(x, dtype=np.float32)
    W_q = np.asarray(W_q, dtype=np.float32)
    W_k = np.asarray(W_k, dtype=np.float32)
    W_v = np.asarray(W_v, dtype=np.float32)
    W_o = np.asarray(W_o, dtype=np.float32)
    if _NC_CACHE is None:
        _NC_CACHE = build()
    nc = _NC_CACHE

    Wvo = W_v @ W_o  # fp32 host-side fold
    xT = [np.ascontiguousarray(x[g].T) for g in range(2)]
    xT_bf = [t.astype(ml_dtypes.bfloat16) for t in xT]
    in_maps = []
    for c in range(8):
        g, r = divmod(c, 4)
        in_maps.append(
            {
                "x": xT[g],
                "xbf": xT_bf[g],
                "wq": np.ascontiguousarray(W_q[:, 512 * r : 512 * (r + 1)]),
                "wk": np.ascontiguousarray(W_k[:, 512 * r : 512 * (r + 1)]),
                "wv": np.ascontiguousarray(Wvo[:, 512 * r : 512 * (r + 1)]).astype(ml_dtypes.bfloat16),
            }
        )
    res = run_bass_kernel_spmd(nc, in_maps, core_ids=list(range(8)))
    Y = np.empty((2, S, D), dtype=np.float32)
    for c in range(8):
        g, r = divmod(c, 4)
        o = res.results[c]["out"]
        for s_idx in range(4):
            t = 4 * s_idx + r
            Y[g, t * 128 : (t + 1) * 128, :] = o[s_idx * 128 : (s_idx + 1) * 128, :]
    return Y
